# revision 19
# baseline (speedup 1.0000x reference)
"""Bass/Trainium2 kernel for the decomposed LocallyConnected2d layer.

out[b,o,i,j] = sum_{c,k} x[b, c, i+di, j+dj] * w[o, c, i, j, k] + bias[o,i,j]
with k = di*3 + dj (3x3 kernel, stride 1).

v7 strategy. Shard over output rows i across 8 cores (4 rows each).
Per-core traffic: w 2.37 MB fp8, x 0.84 MB fp8, out 2.1 MB fp16.

- Both operands in fp8 e3m4 (w*32, x*2; model rel-err 1.2e-2, measured
  on the full problem): the PE matmul runs fp8xfp8 into fp32 PSUM.
  Host gather divides by 64 and adds the bias.
- x: per output row i a tile xt_i [96, 34, 128] (partition di*32+c =
  image row i+di). xt0 is DMA'd in 3 per-di pieces; rows 3..5 come in
  one 32-descriptor DMA (xfr) and DVE shift copies build xt1..xt3
  (xt_i[0:64] <- xt_{i-1}[32:96] in 32-partition pieces, xt_i[64:96]
  <- xfr row i+2). The dj shift is a free-dim offset (j+dj).
- SDMA engines round-robin bandwidth per active queue (~216 GB/s
  total): x rides both HWDGE rings (done ~12 us), w rides the GpSimd
  SWDGE queue as 4 per-row DMAs (w0 lands ~13 us, w3 ~20 us, each just
  ahead of its row's matmuls), outputs follow on the same SWDGE queue.
- Exactly 8 input DMAs -> each gets a fresh completion-semaphore lane
  (9+ would re-raise wait thresholds of everything scheduled later and
  stall the first matmuls on the last w transfer).
"""

import sys

for _p in ("/opt/trn_rl_repo", "/root/.axon_site/_ro/trn_rl_repo"):
    if _p not in sys.path:
        sys.path.append(_p)

import numpy as np

B = 128
C_IN = 32
C_OUT = 64
OH = OW = 32
KH = KW = 3
H = W = 34
N_CORES = 8
RPC = OH // N_CORES          # output rows per core = 4
HALO = RPC + KH - 1          # x rows per core = 6
NPAIR = OW // 2              # j-pairs per row = 16
NGRP = 4                     # j-pairs per psum group
GRPS = NPAIR // NGRP         # psum groups per row = 4

WSCALE = 32.0                # weight scale into fp8 e3m4
XSCALE = 2.0                 # x scale into fp8 e3m4

_prog_cache = {}


def _build_program():
    import concourse.tile as tile
    from concourse import bacc, mybir

    f16 = mybir.dt.float16
    f8 = mybir.dt.float8e3
    f32 = mybir.dt.float32

    nc = bacc.Bacc("TRN2", target_bir_lowering=False, debug=False,
                   num_devices=N_CORES)

    # Per-core DRAM I/O (host pre-sharded / pre-transposed):
    #   x_in [c=32, h=6, w=34, b=128] f8e3 (*2)   halo slice, b innermost
    #   w_in [i=4, p=96, dj=3, j=32, o=64] f8e3 (*32)  p = di*32 + c
    #   out  [p2=128 (par*64+o), i=4, jh=16, b=128] f16 ; j = 2*jh + par
    x_in = nc.dram_tensor("x", [C_IN, HALO, W, B], f8,
                          kind="ExternalInput").ap()
    w_in = nc.dram_tensor("w", [RPC, 96, KW, OW, C_OUT], f8,
                          kind="ExternalInput").ap()
    out = nc.dram_tensor("out", [128, RPC, NPAIR, B], f16,
                         kind="ExternalOutput").ap()

    with tile.TileContext(nc) as tc:
        with (
            tc.tile_pool(name="xpool", bufs=1) as xpool,
            tc.tile_pool(name="wpool", bufs=1) as wpool,
            tc.tile_pool(name="opool", bufs=3) as opool,
            tc.tile_pool(name="pspool", bufs=6, space="PSUM") as pspool,
        ):
            # xt[i] partition di*32+c holds x image row i+di.
            xt = [xpool.tile([96, W, B], f8, tag=f"xt{i}",
                             name=f"xt{i}") for i in range(RPC)]
            # wt[i] partition p, free (dj, j, o).
            wt = [wpool.tile([96, KW, OW, C_OUT], f8, tag=f"wt{i}",
                             name=f"wt{i}") for i in range(RPC)]
            # fresh image rows 3..5, one 32-descriptor DMA (26 KB runs)
            xfr = xpool.tile([C_IN, KH, W, B], f8, tag="xfr", name="xfr")

            # x on both HWDGE rings; per-di 32-partition DMAs keep c
            # (32-wide) as the outermost AP dim so packets spread across
            # all 16 SDMA engines.
            nc.sync.dma_start(xt[0][0:32, :, :], x_in[:, 0])
            nc.sync.dma_start(xt[0][32:64, :, :], x_in[:, 1])
            nc.sync.dma_start(xt[0][64:96, :, :], x_in[:, 2])
            nc.sync.dma_start(wt[0][:], w_in[0])
            nc.sync.dma_start(xfr[:], x_in[:, KH:HALO])
            for i in range(1, RPC):
                nc.sync.dma_start(wt[i][:], w_in[i])

            # Shift copies on DVE, in 32-partition-aligned pieces
            # (engine APs must not cross 32-partition groups from a
            # non-zero start). Flat, not chained: image row r lives in
            # xt0 group r (r<=2) or xfr[r-3] (r>=3), so every copy
            # depends only on the xt0/xfr DMAs and they all run at once.
            u32 = mybir.dt.uint32

            def xrow(r):
                if r <= 2:
                    return xt[0][32 * r:32 * r + 32, :, :]
                return xfr[:, r - 3, :, :]

            for i in range(1, RPC):
                for di in range(KH):
                    nc.vector.tensor_copy(
                        xt[i][32 * di:32 * di + 32, :, :].bitcast(u32),
                        xrow(i + di).bitcast(u32))

            for i in range(RPC):
                out_row = opool.tile([128, NPAIR, B], f16, tag="op")
                for g in range(GRPS):
                    ps = pspool.tile([128, NGRP, B], f32)
                    for pig in range(NGRP):
                        for par in range(2):
                            j = 2 * (NGRP * g + pig) + par
                            pslice = ps[64 * par:64 * par + 64, pig, :]
                            tp = (0, 64 * par)
                            for dj in range(KW):
                                nc.tensor.matmul(pslice, wt[i][:, dj, j, :],
                                                 xt[i][0:96, j + dj, :],
                                                 start=(dj == 0),
                                                 stop=(dj == KW - 1),
                                                 tile_position=tp)
                    dst = out_row[:, NGRP * g:NGRP * (g + 1), :]
                    nc.scalar.copy(dst, ps[:])
                    if i == RPC - 1 and g == 1:
                        # overlap the tail: first half of the last row
                        # leaves as soon as its copies are done
                        nc.gpsimd.dma_start(out[:, i, 0:2 * NGRP, :],
                                            out_row[:, 0:2 * NGRP, :])
                if i == RPC - 1:
                    nc.gpsimd.dma_start(out[:, i, 2 * NGRP:, :],
                                        out_row[:, 2 * NGRP:, :])
                else:
                    nc.gpsimd.dma_start(out[:, i, :, :], out_row[:])

    nc.compile()
    return nc


def _host_prep(x, weight):
    """Full fp32 inputs -> list of per-core input dicts."""
    import ml_dtypes

    # x: (B, C, H, W) -> (C, H, W, B) fp8 e3m4, * 2
    x_t = np.clip(x.transpose(1, 2, 3, 0) * XSCALE, -15.0, 15.0)
    x_8 = x_t.astype(ml_dtypes.float8_e3m4)
    # w: (O, C, I, J, KH, KW) -> [i, p=(di*32+c), dj, j, o] * 32 in e3m4
    w_r = weight.reshape(C_OUT, C_IN, OH, OW, KH, KW)
    w_t = w_r.transpose(2, 4, 1, 5, 3, 0).reshape(OH, 96, KW, OW, C_OUT)
    w_8 = np.clip(w_t * WSCALE, -15.0, 15.0).astype(ml_dtypes.float8_e3m4)

    in_maps = []
    for m in range(N_CORES):
        r0 = m * RPC
        in_maps.append({
            "x": np.ascontiguousarray(x_8[:, r0:r0 + HALO]),
            "w": np.ascontiguousarray(w_8[r0:r0 + RPC]),
        })
    return in_maps


def _gather(results, bias):
    out_full = np.empty((B, C_OUT, OH, OW), np.float32)
    for m in range(N_CORES):
        r = results[m]["out"].astype(np.float32)          # (128, 4, 16, 128)
        r = r.reshape(2, C_OUT, RPC, NPAIR, B)            # par,o,i,jh,b
        r = r.transpose(4, 1, 2, 3, 0)                    # b,o,i,jh,par
        out_full[:, :, m * RPC:(m + 1) * RPC, :] = r.reshape(B, C_OUT, RPC, OW)
    out_full *= 1.0 / (WSCALE * XSCALE)
    out_full += bias[None]
    return out_full


def kernel(x, weight, bias, _trace=False):
    from concourse.bass_utils import run_bass_kernel_spmd

    if "nc" not in _prog_cache:
        _prog_cache["nc"] = _build_program()
    nc = _prog_cache["nc"]

    in_maps = _host_prep(np.asarray(x), np.asarray(weight))
    res = run_bass_kernel_spmd(nc, in_maps, core_ids=list(range(N_CORES)),
                               trace=_trace)
    out = _gather(res.results, np.asarray(bias, np.float32))
    if _trace:
        _prog_cache["last_result"] = res
    return out


# revision 20
# speedup vs baseline: 1.0440x; 1.0440x over previous
"""Bass/Trainium2 kernel for the decomposed LocallyConnected2d layer.

out[b,o,i,j] = sum_{c,k} x[b, c, i+di, j+dj] * w[o, c, i, j, k] + bias[o,i,j]
with k = di*3 + dj (3x3 kernel, stride 1).

v7 strategy. Shard over output rows i across 8 cores (4 rows each).
Per-core traffic: w 2.37 MB fp8, x 0.84 MB fp8, out 2.1 MB fp16.

- Both operands in fp8 e3m4 (w*32, x*2; model rel-err 1.2e-2, measured
  on the full problem): the PE matmul runs fp8xfp8 into fp32 PSUM.
  Host gather divides by 64 and adds the bias.
- x: per output row i a tile xt_i [96, 34, 128] (partition di*32+c =
  image row i+di). xt0 is DMA'd in 3 per-di pieces; rows 3..5 come in
  one 32-descriptor DMA (xfr) and DVE shift copies build xt1..xt3
  (xt_i[0:64] <- xt_{i-1}[32:96] in 32-partition pieces, xt_i[64:96]
  <- xfr row i+2). The dj shift is a free-dim offset (j+dj).
- SDMA engines round-robin bandwidth per active queue (~216 GB/s
  total): x rides both HWDGE rings (done ~12 us), w rides the GpSimd
  SWDGE queue as 4 per-row DMAs (w0 lands ~13 us, w3 ~20 us, each just
  ahead of its row's matmuls), outputs follow on the same SWDGE queue.
- Exactly 8 input DMAs -> each gets a fresh completion-semaphore lane
  (9+ would re-raise wait thresholds of everything scheduled later and
  stall the first matmuls on the last w transfer).
"""

import sys

for _p in ("/opt/trn_rl_repo", "/root/.axon_site/_ro/trn_rl_repo"):
    if _p not in sys.path:
        sys.path.append(_p)

import numpy as np

B = 128
C_IN = 32
C_OUT = 64
OH = OW = 32
KH = KW = 3
H = W = 34
N_CORES = 8
RPC = OH // N_CORES          # output rows per core = 4
HALO = RPC + KH - 1          # x rows per core = 6
NPAIR = OW // 2              # j-pairs per row = 16
NGRP = 4                     # j-pairs per psum group
GRPS = NPAIR // NGRP         # psum groups per row = 4

WSCALE = 32.0                # weight scale into fp8 e3m4
XSCALE = 2.0                 # x scale into fp8 e3m4

_prog_cache = {}


def _build_program():
    import concourse.tile as tile
    from concourse import bacc, mybir

    f16 = mybir.dt.float16
    f8 = mybir.dt.float8e3
    f32 = mybir.dt.float32

    nc = bacc.Bacc("TRN2", target_bir_lowering=False, debug=False,
                   num_devices=N_CORES)

    # Per-core DRAM I/O (host pre-sharded / pre-transposed):
    #   x_in [c=32, h=6, w=34, b=128] f8e3 (*2)   halo slice, b innermost
    #   w_in [i=4, p=96, dj=3, j=32, o=64] f8e3 (*32)  p = di*32 + c
    #   out  [p2=128 (par*64+o), i=4, jh=16, b=128] f16 ; j = 2*jh + par
    x_in = nc.dram_tensor("x", [C_IN, HALO, W, B], f8,
                          kind="ExternalInput").ap()
    w_in = nc.dram_tensor("w", [RPC, 96, KW, OW, C_OUT], f8,
                          kind="ExternalInput").ap()
    out = nc.dram_tensor("out", [128, RPC, NPAIR, B], f16,
                         kind="ExternalOutput").ap()

    with tile.TileContext(nc) as tc:
        with (
            tc.tile_pool(name="xpool", bufs=1) as xpool,
            tc.tile_pool(name="wpool", bufs=1) as wpool,
            tc.tile_pool(name="opool", bufs=3) as opool,
            tc.tile_pool(name="pspool", bufs=6, space="PSUM") as pspool,
        ):
            # xt[i] partition di*32+c holds x image row i+di.
            xt = [xpool.tile([96, W, B], f8, tag=f"xt{i}",
                             name=f"xt{i}") for i in range(RPC)]
            # wt[i] partition p, free (dj, j, o).
            wt = [wpool.tile([96, KW, OW, C_OUT], f8, tag=f"wt{i}",
                             name=f"wt{i}") for i in range(RPC)]
            # fresh image rows 3..5, one 32-descriptor DMA (26 KB runs)
            xfr = xpool.tile([C_IN, KH, W, B], f8, tag="xfr", name="xfr")

            # x on both HWDGE rings; per-di 32-partition DMAs keep c
            # (32-wide) as the outermost AP dim so packets spread across
            # all 16 SDMA engines.
            nc.sync.dma_start(xt[0][0:32, :, :], x_in[:, 0])
            nc.sync.dma_start(xt[0][32:64, :, :], x_in[:, 1])
            nc.sync.dma_start(xt[0][64:96, :, :], x_in[:, 2])
            nc.sync.dma_start(wt[0][:], w_in[0])
            nc.sync.dma_start(xfr[:], x_in[:, KH:HALO])
            for i in range(1, RPC):
                nc.sync.dma_start(wt[i][:], w_in[i])

            # Shift copies on DVE, in 32-partition-aligned pieces
            # (engine APs must not cross 32-partition groups from a
            # non-zero start). Flat, not chained: image row r lives in
            # xt0 group r (r<=2) or xfr[r-3] (r>=3), so every copy
            # depends only on the xt0/xfr DMAs and they all run at once.
            u32 = mybir.dt.uint32

            def xrow(r):
                if r <= 2:
                    return xt[0][32 * r:32 * r + 32, :, :]
                return xfr[:, r - 3, :, :]

            for i in range(1, RPC):
                for di in range(KH):
                    nc.vector.tensor_copy(
                        xt[i][32 * di:32 * di + 32, :, :].bitcast(u32),
                        xrow(i + di).bitcast(u32))

            for i in range(RPC):
                out_row = opool.tile([128, NPAIR, B], f16, tag="op")
                for g in range(GRPS):
                    ps = pspool.tile([128, NGRP, B], f32)
                    for pig in range(NGRP):
                        for par in range(2):
                            j = 2 * (NGRP * g + pig) + par
                            pslice = ps[64 * par:64 * par + 64, pig, :]
                            tp = (0, 64 * par)
                            for dj in range(KW):
                                nc.tensor.matmul(pslice, wt[i][:, dj, j, :],
                                                 xt[i][0:96, j + dj, :],
                                                 start=(dj == 0),
                                                 stop=(dj == KW - 1),
                                                 tile_position=tp)
                    dst = out_row[:, NGRP * g:NGRP * (g + 1), :]
                    nc.scalar.copy(dst, ps[:])
                nc.gpsimd.dma_start(out[:, i, :, :], out_row[:])

    nc.compile()
    return nc


def _host_prep(x, weight):
    """Full fp32 inputs -> list of per-core input dicts."""
    import ml_dtypes

    # x: (B, C, H, W) -> (C, H, W, B) fp8 e3m4, * 2
    x_t = np.clip(x.transpose(1, 2, 3, 0) * XSCALE, -15.0, 15.0)
    x_8 = x_t.astype(ml_dtypes.float8_e3m4)
    # w: (O, C, I, J, KH, KW) -> [i, p=(di*32+c), dj, j, o] * 32 in e3m4
    w_r = weight.reshape(C_OUT, C_IN, OH, OW, KH, KW)
    w_t = w_r.transpose(2, 4, 1, 5, 3, 0).reshape(OH, 96, KW, OW, C_OUT)
    w_8 = np.clip(w_t * WSCALE, -15.0, 15.0).astype(ml_dtypes.float8_e3m4)

    in_maps = []
    for m in range(N_CORES):
        r0 = m * RPC
        in_maps.append({
            "x": np.ascontiguousarray(x_8[:, r0:r0 + HALO]),
            "w": np.ascontiguousarray(w_8[r0:r0 + RPC]),
        })
    return in_maps


def _gather(results, bias):
    out_full = np.empty((B, C_OUT, OH, OW), np.float32)
    for m in range(N_CORES):
        r = results[m]["out"].astype(np.float32)          # (128, 4, 16, 128)
        r = r.reshape(2, C_OUT, RPC, NPAIR, B)            # par,o,i,jh,b
        r = r.transpose(4, 1, 2, 3, 0)                    # b,o,i,jh,par
        out_full[:, :, m * RPC:(m + 1) * RPC, :] = r.reshape(B, C_OUT, RPC, OW)
    out_full *= 1.0 / (WSCALE * XSCALE)
    out_full += bias[None]
    return out_full


def kernel(x, weight, bias, _trace=False):
    from concourse.bass_utils import run_bass_kernel_spmd

    if "nc" not in _prog_cache:
        _prog_cache["nc"] = _build_program()
    nc = _prog_cache["nc"]

    in_maps = _host_prep(np.asarray(x), np.asarray(weight))
    res = run_bass_kernel_spmd(nc, in_maps, core_ids=list(range(N_CORES)),
                               trace=_trace)
    out = _gather(res.results, np.asarray(bias, np.float32))
    if _trace:
        _prog_cache["last_result"] = res
    return out
